# revision 25
# baseline (speedup 1.0000x reference)
"""ContextQueryAttention (BiDAF-style) Trainium2 kernel, 8-core data-parallel.

Math (per batch):
  s[i,j]  = wq.q_j + wc.c_i + sum_d c_id * wcq_d * q_jd          (L1 x L2)
  s1      = softmax_i(s * mq_j + (1-mq_j)*NEG)                   (softmax over i)
  s2      = softmax_i(s * mp_i + (1-mp_i)*NEG)
  a       = s1 @ Q                 (L1 x D)
  b       = (s1 @ s2^T) @ C  ==  s1 @ (s2^T @ C)   <- reassociated, no L1xL1
  out     = [C, a, C*a, C*b]                                      (L1 x 4D)

Key kernel facts:
 - scores ~ N(0,1): no max-subtraction needed for a stable softmax.
 - qwq_j is constant along the softmax axis (i) in both softmaxes, so it
   cancels in s1 and s2 entirely and is never computed.
 - E1 path, ST layout [j part, i free]: E1 = exp(mq_j*(dot+cwc_i+1000) -
   1000*mq_j); cwc_i+1000 added in f32 via a partition-broadcast row (bf16
   would quantize +-2 at magnitude 1000); masked col -> exp(0)=1 -> uniform
   1/L1, exactly matching the reference. Z1 via ACT accum_out.
 - E2 path, natural layout [i part, j free] from a 2nd score matmul:
   E2 = exp(mp_i*dot + (mp_i*(cwc_i+1000) - 1000)) fully fused in one ACT op
   (per-partition scale+bias); masked entries underflow to exactly 0.
   Z2 via a ones-column appended to C in the t matmul.
 - matmul operands bf16 (fp32 matmul runs as 2 HW passes + slow LDWEIGHTS),
   accumulation f32 in PSUM.
 - the out[:, 0:128] = context section is a direct DRAM->DRAM DMA.
 - the two per-core batches are emitted phase-interleaved so the Tile
   scheduler always has independent work adjacent to any stalled chain.
"""

import numpy as np

import concourse.bass as bass
import concourse.mybir as mybir
import concourse.tile as tile
from concourse import bacc
from concourse import bass_utils
from concourse.masks import make_identity

F32 = mybir.dt.float32
BF16 = mybir.dt.bfloat16
EXP = mybir.ActivationFunctionType.Exp
IDENT = mybir.ActivationFunctionType.Identity
ADD = mybir.AluOpType.add
MULT = mybir.AluOpType.mult

B, L1, L2, D = 16, 2048, 512, 128
NCORES = 8
BPC = B // NCORES          # batches per core
NT1 = L1 // 128            # 16 i-tiles
NT2 = L2 // 128            # 4  j-tiles
SHIFT = 1000.0             # makes masked E2 entries underflow exp to 0.0


def _build_program(dbg=False):
    nc = bacc.Bacc("TRN2", target_bir_lowering=False, debug=False)

    ctx_d = nc.dram_tensor("context", [BPC, L1, D], F32, kind="ExternalInput").ap()
    qry_d = nc.dram_tensor("query", [BPC, L2, D], F32, kind="ExternalInput").ap()
    w_d = nc.dram_tensor("w", [3, D], F32, kind="ExternalInput").ap()
    mp_d = nc.dram_tensor("mask_p", [BPC, L1], F32, kind="ExternalInput").ap()
    mq_d = nc.dram_tensor("mask_q", [BPC, L2], F32, kind="ExternalInput").ap()
    out_d = nc.dram_tensor("out", [BPC, L1, 4 * D], F32, kind="ExternalOutput").ap()

    with tile.TileContext(nc) as tc:
        with (
            tc.tile_pool(name="const", bufs=1) as const,
            tc.tile_pool(name="big", bufs=2) as big,
            tc.tile_pool(name="work", bufs=2) as work,
            tc.tile_pool(name="outp", bufs=4) as outp,
            tc.tile_pool(name="ps512", bufs=2, space="PSUM") as ps512,
            tc.tile_pool(name="ps256", bufs=4, space="PSUM") as ps256,
            tc.tile_pool(name="psrow", bufs=2, space="PSUM") as psrow,
            tc.tile_pool(name="dramp", bufs=2, space="DRAM") as dramp,
        ):
            ident_b = const.tile([128, 128], BF16)
            make_identity(nc, ident_b)
            w_sb = const.tile([128, 3], F32)  # cols: wq, wc, wcq
            nc.sync.dma_start(out=w_sb, in_=w_d.rearrange("k d -> d k"))
            w_b = const.tile([128, 3], BF16)
            nc.vector.tensor_copy(w_b, w_sb)
            shift_col = const.tile([128, 1], F32)
            nc.vector.memset(shift_col, SHIFT)

            S = [dict() for _ in range(BPC)]  # per-batch tile state

            def ph_dma(b):
                s = S[b]
                s["qn"] = work.tile([128, NT2, 128], F32, tag="qn", name=f"qn{b}")
                nc.sync.dma_start(
                    out=s["qn"], in_=qry_d[b].rearrange("(t p) d -> p t d", p=128)
                )
                s["mp"] = work.tile([128, NT1], F32, tag="mp", name=f"mp{b}")
                nc.sync.dma_start(
                    out=s["mp"], in_=mp_d[b].rearrange("(t p) -> p t", p=128)
                )
                s["mq"] = work.tile([128, NT2], F32, tag="mq", name=f"mq{b}")
                nc.sync.dma_start(
                    out=s["mq"], in_=mq_d[b].rearrange("(t p) -> p t", p=128)
                )
                s["c1"] = big.tile([128, NT1, 128], F32, tag="c1", name=f"c1_{b}")
                ctx_r = ctx_d[b].rearrange("(t p) d -> p t d", p=128)
                for n in range(4):
                    eng = nc.sync if n % 2 == 0 else nc.scalar
                    eng.dma_start(
                        out=s["c1"][:, 4 * n : 4 * (n + 1), :],
                        in_=ctx_r[:, 4 * n : 4 * (n + 1), :],
                    )

            def ph_qside(b):
                s = S[b]
                qnb = work.tile([128, NT2, 128], BF16, tag="qnb")
                for jt in range(NT2):
                    nc.any.tensor_copy(qnb[:, jt, :], s["qn"][:, jt, :])
                s["qnb"] = qnb
                qt = work.tile([128, NT2, 128], BF16, tag="qt")
                ps = ps256.tile([128, 4, 128], BF16, tag="acc")
                for jt in range(NT2):
                    nc.tensor.transpose(ps[:, jt, :], qnb[:, jt, :], ident_b)
                nc.any.tensor_copy(qt, ps)
                s["qt"] = qt
                qtw = work.tile([128, NT2, 128], BF16, tag="qtw")
                nc.vector.tensor_scalar_mul(qtw, qt, w_sb[:, 2:3])
                s["qtw"] = qtw

            def ph_cside(b):
                s = S[b]
                c1b = big.tile([128, NT1, 129], BF16, tag="c1b")
                for it in range(NT1):
                    eng = nc.any if it % 2 == 0 else nc.gpsimd
                    eng.tensor_copy(c1b[:, it, 0:128], s["c1"][:, it, :])
                nc.vector.memset(c1b[:, :, 128:129], 1.0)
                s["c1b"] = c1b
                ct = big.tile([128, NT1, 128], BF16, tag="ct")
                for n in range(4):
                    ps = ps256.tile([128, 4, 128], BF16, tag="acc")
                    for k in range(4):
                        nc.tensor.transpose(
                            ps[:, k, :], c1b[:, 4 * n + k, 0:128], ident_b
                        )
                    nc.any.tensor_copy(ct[:, 4 * n : 4 * (n + 1), :], ps)
                s["ct"] = ct
                cwt = big.tile([128, NT1, 128], BF16, tag="cwt")
                nc.vector.tensor_scalar_mul(cwt, ct, w_sb[:, 2:3])
                s["cwt"] = cwt

            def ph_bias(b):
                s = S[b]
                # cwc_nat (raw cwc) via 16 tiny matmuls, no DRAM roundtrip
                cwc_nat = work.tile([128, NT1], F32, tag="cwc_nat", name=f"cwn{b}")
                for it in range(NT1):
                    psc = ps256.tile([128, 1], F32, tag="acc", name=f"psc{b}_{it}")
                    nc.tensor.matmul(
                        psc, s["ct"][:, it, :], w_b[:, 1:2], start=True, stop=True
                    )
                    nc.any.tensor_copy(cwc_nat[:, it : it + 1], psc)
                bias1 = work.tile([128, NT2], F32, tag="bias1", name=f"b1{b}")
                nc.vector.tensor_scalar_mul(bias1, s["mq"], -SHIFT)
                s["bias1"] = bias1
                bias2 = work.tile([128, NT1], F32, tag="bias2", name=f"b2{b}")
                nc.vector.scalar_tensor_tensor(
                    out=bias2, in0=cwc_nat, scalar=SHIFT, in1=s["mp"],
                    op0=ADD, op1=MULT,
                )
                nc.vector.tensor_scalar_add(bias2, bias2, -SHIFT)
                s["bias2"] = bias2

            def ph_cwcrow(b):
                s = S[b]
                cwc_row = work.tile([1, L1], F32, tag="cwc_row", name=f"cwr{b}")
                cwc_bc = big.tile([128, L1], F32, tag="cwc_bc", name=f"cwb{b}")
                for n in range(4):
                    psr = psrow.tile([1, 512], F32, tag="cwcr", name=f"psr{b}_{n}")
                    nc.tensor.matmul(
                        psr, w_b[:, 1:2], s["ct"][:, 4 * n : 4 * (n + 1), :],
                        start=True, stop=True,
                    )
                    nc.scalar.activation(
                        cwc_row[:, 512 * n : 512 * (n + 1)], psr, IDENT,
                        bias=shift_col[0:1, :],
                    )
                    nc.gpsimd.partition_broadcast(
                        cwc_bc[:, 512 * n : 512 * (n + 1)],
                        cwc_row[:, 512 * n : 512 * (n + 1)],
                    )
                s["cwc_bc"] = cwc_bc

            def ph_e1(b):
                s = S[b]
                e1 = big.tile([128, NT2, L1], BF16, tag="e1")
                z1 = work.tile([128, NT2], F32, tag="z1")
                for jt in range(NT2):
                    st_sb = work.tile([128, L1], F32, tag="st_sb")
                    for n in range(4):
                        psst = ps512.tile([128, 512], F32, tag="mm512")
                        nc.tensor.matmul(
                            psst, s["qtw"][:, jt, :],
                            s["ct"][:, 4 * n : 4 * (n + 1), :],
                            start=True, stop=True,
                        )
                        nc.vector.tensor_tensor(
                            st_sb[:, 512 * n : 512 * (n + 1)], psst,
                            s["cwc_bc"][:, 512 * n : 512 * (n + 1)], ADD,
                        )
                    nc.scalar.activation(
                        e1[:, jt, :], st_sb, EXP,
                        bias=s["bias1"][:, jt : jt + 1],
                        scale=s["mq"][:, jt : jt + 1],
                        accum_out=z1[:, jt : jt + 1],
                    )
                s["e1"], s["z1"] = e1, z1

            def ph_e2(b):
                s = S[b]
                e2n = big.tile([128, NT1, L2], BF16, tag="e2n")
                for it in range(NT1):
                    pss = ps512.tile([128, 512], F32, tag="mm512")
                    nc.tensor.matmul(pss, s["cwt"][:, it, :], s["qt"], start=True, stop=True)
                    nc.scalar.activation(
                        e2n[:, it, :], pss, EXP,
                        bias=s["bias2"][:, it : it + 1],
                        scale=s["mp"][:, it : it + 1],
                    )
                s["e2n"] = e2n

            def ph_t(b):
                s = S[b]
                rz1 = work.tile([128, NT2], F32, tag="rz1")
                nc.vector.reciprocal(rz1, s["z1"])
                rhs_ab = work.tile([128, NT2, 256], BF16, tag="rhs_ab")
                for jt in range(NT2):
                    pst = ps256.tile([128, 129], F32, tag="acc")
                    for it in range(NT1):
                        nc.tensor.matmul(
                            pst, s["e2n"][:, it, jt * 128 : (jt + 1) * 128],
                            s["c1b"][:, it, :],
                            start=(it == 0), stop=(it == NT1 - 1),
                        )
                    rz2 = work.tile([128, 1], F32, tag="rz2")
                    nc.vector.reciprocal(rz2, pst[:, 128:129])
                    rz12 = work.tile([128, 1], F32, tag="rz12")
                    nc.vector.tensor_mul(rz12, rz2, rz1[:, jt : jt + 1])
                    nc.vector.tensor_scalar_mul(
                        rhs_ab[:, jt, 128:256], pst[:, 0:128], rz12
                    )
                    nc.vector.tensor_scalar_mul(
                        rhs_ab[:, jt, 0:128], s["qnb"][:, jt, :], rz1[:, jt : jt + 1]
                    )
                s["rhs_ab"] = rhs_ab

            def ph_ab(b):
                s = S[b]
                for it in range(NT1):
                    psab = ps256.tile([128, 256], F32, tag="acc")
                    for jt in range(NT2):
                        nc.tensor.matmul(
                            psab,
                            s["e1"][:, jt, it * 128 : (it + 1) * 128],
                            s["rhs_ab"][:, jt, :],
                            start=(jt == 0), stop=(jt == NT2 - 1),
                        )
                    o_sb = outp.tile([128, 384], F32, tag="o_sb")
                    nc.any.tensor_copy(o_sb[:, 0:128], psab[:, 0:128])
                    nc.vector.tensor_mul(
                        o_sb[:, 128:256], s["c1"][:, it, :], psab[:, 0:128]
                    )
                    nc.vector.tensor_mul(
                        o_sb[:, 256:384], s["c1"][:, it, :], psab[:, 128:256]
                    )
                    nc.sync.dma_start(
                        out=out_d[b, it * 128 : (it + 1) * 128, 128:512], in_=o_sb
                    )

            def ph_dbg(b):
                if not (dbg and b == 0):
                    return
                s = S[b]
                for name, key in [
                    ("dbg_e1", "e1"), ("dbg_e2n", "e2n"), ("dbg_z1", "z1"),
                    ("dbg_bias2", "bias2"), ("dbg_rhs_ab", "rhs_ab"),
                    ("dbg_ct", "ct"), ("dbg_qt", "qt"),
                ]:
                    src = s[key]
                    dd = nc.dram_tensor(
                        name, list(src.shape), src.dtype, kind="ExternalOutput"
                    ).ap()
                    nc.sync.dma_start(out=dd, in_=src)

            # interleaved emission: scheduler always has cross-batch slack
            ph_dma(0); ph_qside(0); ph_dma(1); ph_cside(0); ph_qside(1)
            ph_bias(0); ph_cside(1); ph_e2(0); ph_bias(1); ph_cwcrow(0)
            ph_e2(1); ph_e1(0); ph_cwcrow(1); ph_t(0); ph_e1(1)
            ph_ab(0); ph_t(1); ph_ab(1)
            # out[:, :, 0:128] = context, straight DRAM->DRAM (no deps; last)
            for b in range(BPC):
                nc.scalar.dma_start(out=out_d[b, :, 0:128], in_=ctx_d[b])
            ph_dbg(0)

    nc.compile()
    return nc


_NC = None


def _get_nc():
    global _NC
    if _NC is None:
        _NC = _build_program()
    return _NC


def _make_in_maps(inputs):
    context, query, w = inputs["context"], inputs["query"], inputs["w"]
    w2 = np.ascontiguousarray(np.asarray(w).reshape(3, D).astype(np.float32))
    mp = np.asarray(inputs["mask_p"]).astype(np.float32)
    mq = np.asarray(inputs["mask_q"]).astype(np.float32)
    in_maps = []
    for c in range(NCORES):
        sl = slice(c * BPC, (c + 1) * BPC)
        in_maps.append(
            {
                "context": np.ascontiguousarray(context[sl]),
                "query": np.ascontiguousarray(query[sl]),
                "w": w2,
                "mask_p": np.ascontiguousarray(mp[sl]),
                "mask_q": np.ascontiguousarray(mq[sl]),
            }
        )
    return in_maps


def kernel(context, query, w, mask_p, mask_q):
    nc = _get_nc()
    in_maps = _make_in_maps(
        {"context": context, "query": query, "w": w, "mask_p": mask_p, "mask_q": mask_q}
    )
    res = bass_utils.run_bass_kernel_spmd(nc, in_maps, core_ids=list(range(NCORES)))
    return np.concatenate([res.results[c]["out"] for c in range(NCORES)], axis=0)


# revision 26
# speedup vs baseline: 1.0158x; 1.0158x over previous
"""ContextQueryAttention (BiDAF-style) Trainium2 kernel, 8-core data-parallel.

Math (per batch):
  s[i,j]  = wq.q_j + wc.c_i + sum_d c_id * wcq_d * q_jd          (L1 x L2)
  s1      = softmax_i(s * mq_j + (1-mq_j)*NEG)                   (softmax over i)
  s2      = softmax_i(s * mp_i + (1-mp_i)*NEG)
  a       = s1 @ Q                 (L1 x D)
  b       = (s1 @ s2^T) @ C  ==  s1 @ (s2^T @ C)   <- reassociated, no L1xL1
  out     = [C, a, C*a, C*b]                                      (L1 x 4D)

Key kernel facts:
 - scores ~ N(0,1): no max-subtraction needed for a stable softmax.
 - qwq_j is constant along the softmax axis (i) in both softmaxes, so it
   cancels in s1 and s2 entirely and is never computed.
 - E1 path, ST layout [j part, i free]: E1 = exp(mq_j*(dot+cwc_i+1000) -
   1000*mq_j); cwc_i+1000 added in f32 via a partition-broadcast row (bf16
   would quantize +-2 at magnitude 1000); masked col -> exp(0)=1 -> uniform
   1/L1, exactly matching the reference. Z1 via ACT accum_out.
 - E2 path, natural layout [i part, j free] from a 2nd score matmul:
   E2 = exp(mp_i*dot + (mp_i*(cwc_i+1000) - 1000)) fully fused in one ACT op
   (per-partition scale+bias); masked entries underflow to exactly 0.
   Z2 via a ones-column appended to C in the t matmul.
 - matmul operands bf16 (fp32 matmul runs as 2 HW passes + slow LDWEIGHTS),
   accumulation f32 in PSUM.
 - the out[:, 0:128] = context section is a direct DRAM->DRAM DMA.
 - the two per-core batches are emitted phase-interleaved so the Tile
   scheduler always has independent work adjacent to any stalled chain.
"""

import numpy as np

import concourse.bass as bass
import concourse.mybir as mybir
import concourse.tile as tile
from concourse import bacc
from concourse import bass_utils
from concourse.masks import make_identity

F32 = mybir.dt.float32
BF16 = mybir.dt.bfloat16
EXP = mybir.ActivationFunctionType.Exp
IDENT = mybir.ActivationFunctionType.Identity
ADD = mybir.AluOpType.add
MULT = mybir.AluOpType.mult

B, L1, L2, D = 16, 2048, 512, 128
NCORES = 8
BPC = B // NCORES          # batches per core
NT1 = L1 // 128            # 16 i-tiles
NT2 = L2 // 128            # 4  j-tiles
SHIFT = 1000.0             # makes masked E2 entries underflow exp to 0.0


def _build_program(dbg=False):
    nc = bacc.Bacc("TRN2", target_bir_lowering=False, debug=False)

    ctx_d = nc.dram_tensor("context", [BPC, L1, D], F32, kind="ExternalInput").ap()
    qry_d = nc.dram_tensor("query", [BPC, L2, D], F32, kind="ExternalInput").ap()
    w_d = nc.dram_tensor("w", [3, D], F32, kind="ExternalInput").ap()
    mp_d = nc.dram_tensor("mask_p", [BPC, L1], F32, kind="ExternalInput").ap()
    mq_d = nc.dram_tensor("mask_q", [BPC, L2], F32, kind="ExternalInput").ap()
    out_d = nc.dram_tensor("out", [BPC, L1, 4 * D], F32, kind="ExternalOutput").ap()

    with tile.TileContext(nc) as tc:
        with (
            tc.tile_pool(name="const", bufs=1) as const,
            tc.tile_pool(name="big", bufs=2) as big,
            tc.tile_pool(name="work", bufs=2) as work,
            tc.tile_pool(name="outp", bufs=4) as outp,
            tc.tile_pool(name="ps512", bufs=2, space="PSUM") as ps512,
            tc.tile_pool(name="ps256", bufs=4, space="PSUM") as ps256,
            tc.tile_pool(name="psrow", bufs=2, space="PSUM") as psrow,
            tc.tile_pool(name="dramp", bufs=2, space="DRAM") as dramp,
        ):
            ident_b = const.tile([128, 128], BF16)
            make_identity(nc, ident_b)
            w_sb = const.tile([128, 3], F32)  # cols: wq, wc, wcq
            nc.sync.dma_start(out=w_sb, in_=w_d.rearrange("k d -> d k"))
            w_b = const.tile([128, 3], BF16)
            nc.vector.tensor_copy(w_b, w_sb)
            shift_col = const.tile([128, 1], F32)
            nc.vector.memset(shift_col, SHIFT)

            S = [dict() for _ in range(BPC)]  # per-batch tile state

            def ph_dma(b):
                s = S[b]
                s["qn"] = work.tile([128, NT2, 128], F32, tag="qn", name=f"qn{b}")
                nc.sync.dma_start(
                    out=s["qn"], in_=qry_d[b].rearrange("(t p) d -> p t d", p=128)
                )
                s["mp"] = work.tile([128, NT1], F32, tag="mp", name=f"mp{b}")
                nc.sync.dma_start(
                    out=s["mp"], in_=mp_d[b].rearrange("(t p) -> p t", p=128)
                )
                s["mq"] = work.tile([128, NT2], F32, tag="mq", name=f"mq{b}")
                nc.sync.dma_start(
                    out=s["mq"], in_=mq_d[b].rearrange("(t p) -> p t", p=128)
                )
                s["c1"] = big.tile([128, NT1, 128], F32, tag="c1", name=f"c1_{b}")
                ctx_r = ctx_d[b].rearrange("(t p) d -> p t d", p=128)
                for n in range(4):
                    eng = nc.sync if n % 2 == 0 else nc.scalar
                    eng.dma_start(
                        out=s["c1"][:, 4 * n : 4 * (n + 1), :],
                        in_=ctx_r[:, 4 * n : 4 * (n + 1), :],
                    )

            def ph_qside(b):
                s = S[b]
                qnb = work.tile([128, NT2, 128], BF16, tag="qnb")
                for jt in range(NT2):
                    nc.any.tensor_copy(qnb[:, jt, :], s["qn"][:, jt, :])
                s["qnb"] = qnb
                qt = work.tile([128, NT2, 128], BF16, tag="qt")
                ps = ps256.tile([128, 4, 128], BF16, tag="acc")
                for jt in range(NT2):
                    nc.tensor.transpose(ps[:, jt, :], qnb[:, jt, :], ident_b)
                nc.any.tensor_copy(qt, ps)
                s["qt"] = qt
                qtw = work.tile([128, NT2, 128], BF16, tag="qtw")
                nc.vector.tensor_scalar_mul(qtw, qt, w_sb[:, 2:3])
                s["qtw"] = qtw

            def ph_cside(b):
                s = S[b]
                c1b = big.tile([128, NT1, 129], BF16, tag="c1b")
                for it in range(NT1):
                    eng = nc.any if it % 2 == 0 else nc.gpsimd
                    eng.tensor_copy(c1b[:, it, 0:128], s["c1"][:, it, :])
                nc.vector.memset(c1b[:, :, 128:129], 1.0)
                s["c1b"] = c1b
                ct = big.tile([128, NT1, 128], BF16, tag="ct")
                for n in range(4):
                    ps = ps256.tile([128, 4, 128], BF16, tag="acc")
                    for k in range(4):
                        nc.tensor.transpose(
                            ps[:, k, :], c1b[:, 4 * n + k, 0:128], ident_b
                        )
                    nc.any.tensor_copy(ct[:, 4 * n : 4 * (n + 1), :], ps)
                s["ct"] = ct
                cwt = big.tile([128, NT1, 128], BF16, tag="cwt")
                nc.vector.tensor_scalar_mul(cwt, ct, w_sb[:, 2:3])
                s["cwt"] = cwt

            def ph_bias(b):
                s = S[b]
                # cwc_nat (raw cwc) via 16 tiny matmuls, no DRAM roundtrip
                cwc_nat = work.tile([128, NT1], F32, tag="cwc_nat", name=f"cwn{b}")
                for it in range(NT1):
                    psc = ps256.tile([128, 1], F32, tag="acc", name=f"psc{b}_{it}")
                    nc.tensor.matmul(
                        psc, s["ct"][:, it, :], w_b[:, 1:2], start=True, stop=True
                    )
                    nc.any.tensor_copy(cwc_nat[:, it : it + 1], psc)
                bias1 = work.tile([128, NT2], F32, tag="bias1", name=f"b1{b}")
                nc.vector.tensor_scalar_mul(bias1, s["mq"], -SHIFT)
                s["bias1"] = bias1
                bias2 = work.tile([128, NT1], F32, tag="bias2", name=f"b2{b}")
                nc.vector.scalar_tensor_tensor(
                    out=bias2, in0=cwc_nat, scalar=SHIFT, in1=s["mp"],
                    op0=ADD, op1=MULT,
                )
                nc.vector.tensor_scalar_add(bias2, bias2, -SHIFT)
                s["bias2"] = bias2

            def ph_cwcrow(b):
                s = S[b]
                cwc_row = work.tile([1, L1], F32, tag="cwc_row", name=f"cwr{b}")
                cwc_bc = big.tile([128, L1], F32, tag="cwc_bc", name=f"cwb{b}")
                for n in range(4):
                    psr = psrow.tile([1, 512], F32, tag="cwcr", name=f"psr{b}_{n}")
                    nc.tensor.matmul(
                        psr, w_b[:, 1:2], s["ct"][:, 4 * n : 4 * (n + 1), :],
                        start=True, stop=True,
                    )
                    nc.scalar.activation(
                        cwc_row[:, 512 * n : 512 * (n + 1)], psr, IDENT,
                        bias=shift_col[0:1, :],
                    )
                    nc.gpsimd.partition_broadcast(
                        cwc_bc[:, 512 * n : 512 * (n + 1)],
                        cwc_row[:, 512 * n : 512 * (n + 1)],
                    )
                s["cwc_bc"] = cwc_bc

            def ph_e1(b):
                s = S[b]
                e1 = big.tile([128, NT2, L1], BF16, tag="e1")
                z1 = work.tile([128, NT2], F32, tag="z1")
                for jt in range(NT2):
                    st_sb = work.tile([128, L1], F32, tag="st_sb")
                    for n in range(4):
                        psst = ps512.tile([128, 512], F32, tag="mm512")
                        nc.tensor.matmul(
                            psst, s["qtw"][:, jt, :],
                            s["ct"][:, 4 * n : 4 * (n + 1), :],
                            start=True, stop=True,
                        )
                        nc.vector.tensor_tensor(
                            st_sb[:, 512 * n : 512 * (n + 1)], psst,
                            s["cwc_bc"][:, 512 * n : 512 * (n + 1)], ADD,
                        )
                    nc.scalar.activation(
                        e1[:, jt, :], st_sb, EXP,
                        bias=s["bias1"][:, jt : jt + 1],
                        scale=s["mq"][:, jt : jt + 1],
                        accum_out=z1[:, jt : jt + 1],
                    )
                s["e1"], s["z1"] = e1, z1

            def ph_e2(b):
                s = S[b]
                e2n = big.tile([128, NT1, L2], BF16, tag="e2n")
                for it in range(NT1):
                    pss = ps512.tile([128, 512], F32, tag="mm512")
                    nc.tensor.matmul(pss, s["cwt"][:, it, :], s["qt"], start=True, stop=True)
                    nc.scalar.activation(
                        e2n[:, it, :], pss, EXP,
                        bias=s["bias2"][:, it : it + 1],
                        scale=s["mp"][:, it : it + 1],
                    )
                s["e2n"] = e2n

            def ph_t(b):
                s = S[b]
                rz1 = work.tile([128, NT2], F32, tag="rz1")
                nc.vector.reciprocal(rz1, s["z1"])
                rhs_ab = work.tile([128, NT2, 256], BF16, tag="rhs_ab")
                for jt in range(NT2):
                    pst = ps256.tile([128, 129], F32, tag="acc")
                    for it in range(NT1):
                        nc.tensor.matmul(
                            pst, s["e2n"][:, it, jt * 128 : (jt + 1) * 128],
                            s["c1b"][:, it, :],
                            start=(it == 0), stop=(it == NT1 - 1),
                        )
                    rz2 = work.tile([128, 1], F32, tag="rz2")
                    nc.vector.reciprocal(rz2, pst[:, 128:129])
                    rz12 = work.tile([128, 1], F32, tag="rz12")
                    nc.vector.tensor_mul(rz12, rz2, rz1[:, jt : jt + 1])
                    nc.vector.tensor_scalar_mul(
                        rhs_ab[:, jt, 128:256], pst[:, 0:128], rz12
                    )
                    nc.vector.tensor_scalar_mul(
                        rhs_ab[:, jt, 0:128], s["qnb"][:, jt, :], rz1[:, jt : jt + 1]
                    )
                s["rhs_ab"] = rhs_ab

            def ph_ab(b):
                s = S[b]
                for it in range(NT1):
                    psab = ps256.tile([128, 256], F32, tag="acc")
                    for jt in range(NT2):
                        nc.tensor.matmul(
                            psab,
                            s["e1"][:, jt, it * 128 : (it + 1) * 128],
                            s["rhs_ab"][:, jt, :],
                            start=(jt == 0), stop=(jt == NT2 - 1),
                        )
                    o_sb = outp.tile([128, 384], F32, tag="o_sb")
                    nc.any.tensor_copy(o_sb[:, 0:128], psab[:, 0:128])
                    nc.vector.tensor_mul(
                        o_sb[:, 128:256], s["c1"][:, it, :], psab[:, 0:128]
                    )
                    nc.vector.tensor_mul(
                        o_sb[:, 256:384], s["c1"][:, it, :], psab[:, 128:256]
                    )
                    nc.sync.dma_start(
                        out=out_d[b, it * 128 : (it + 1) * 128, 128:512], in_=o_sb
                    )

            def ph_dbg(b):
                if not (dbg and b == 0):
                    return
                s = S[b]
                for name, key in [
                    ("dbg_e1", "e1"), ("dbg_e2n", "e2n"), ("dbg_z1", "z1"),
                    ("dbg_bias2", "bias2"), ("dbg_rhs_ab", "rhs_ab"),
                    ("dbg_ct", "ct"), ("dbg_qt", "qt"),
                ]:
                    src = s[key]
                    dd = nc.dram_tensor(
                        name, list(src.shape), src.dtype, kind="ExternalOutput"
                    ).ap()
                    nc.sync.dma_start(out=dd, in_=src)

            # interleaved emission: scheduler always has cross-batch slack
            ph_dma(0); ph_qside(0); ph_dma(1); ph_cside(0)
            # out[:, :, 0:128] = context, straight DRAM->DRAM (no deps;
            # on the ACT HWDGE queue which is idle during startup)
            for b in range(BPC):
                nc.scalar.dma_start(out=out_d[b, :, 0:128], in_=ctx_d[b])
            ph_qside(1)
            ph_bias(0); ph_cside(1); ph_e2(0); ph_bias(1); ph_cwcrow(0)
            ph_e2(1); ph_e1(0); ph_cwcrow(1); ph_t(0); ph_e1(1)
            ph_ab(0); ph_t(1); ph_ab(1)
            ph_dbg(0)

    nc.compile()
    return nc


_NC = None


def _get_nc():
    global _NC
    if _NC is None:
        _NC = _build_program()
    return _NC


def _make_in_maps(inputs):
    context, query, w = inputs["context"], inputs["query"], inputs["w"]
    w2 = np.ascontiguousarray(np.asarray(w).reshape(3, D).astype(np.float32))
    mp = np.asarray(inputs["mask_p"]).astype(np.float32)
    mq = np.asarray(inputs["mask_q"]).astype(np.float32)
    in_maps = []
    for c in range(NCORES):
        sl = slice(c * BPC, (c + 1) * BPC)
        in_maps.append(
            {
                "context": np.ascontiguousarray(context[sl]),
                "query": np.ascontiguousarray(query[sl]),
                "w": w2,
                "mask_p": np.ascontiguousarray(mp[sl]),
                "mask_q": np.ascontiguousarray(mq[sl]),
            }
        )
    return in_maps


def kernel(context, query, w, mask_p, mask_q):
    nc = _get_nc()
    in_maps = _make_in_maps(
        {"context": context, "query": query, "w": w, "mask_p": mask_p, "mask_q": mask_q}
    )
    res = bass_utils.run_bass_kernel_spmd(nc, in_maps, core_ids=list(range(NCORES)))
    return np.concatenate([res.results[c]["out"] for c in range(NCORES)], axis=0)
